# revision 5
# baseline (speedup 1.0000x reference)
"""Trainium2 Bass kernel for 2D-relative-bias multi-head attention.

Shapes (hardcoded): x [64, 16, 16, 512], 16 heads x 32 dim, S = 256.
Sharding: data-parallel over batch, 8 batches per core on 8 cores.

Per-core device pipeline (fp32 PSUM accumulation everywhere):
  qT/kT = W^T @ x^T            [nd, tok]   (PE; fp8 DoubleRow K=256 x2, or bf16)
  v     = x @ Wv               [tok, nd]   (PE, bf16)
  logitsT[t,s] per head        (PE, K=32, 4-head row-packed via tile_position)
  E0 = exp(scale*logitsT)      (ACT, PSUM->SBUF bf16) -- the HW critical chain
  E  = E0 * exp(biasT)         (DVE + GPSIMD column-split, bias table from host)
  sums = 1^T E (replicated)    (PE, 4-head col-packed, all-ones lhsT)
  out_unT = V^T E              (PE, 4-head col-packed)
  R = 1/sums                   (DVE reciprocal_approx_fast)
  outT = out_unT * R           (DVE)
  final = outT^T @ Wo + o_b    (PE, bf16)

q/k projections, per-token-chunk tiles: chunk c feeds batches 2c, 2c+1 only,
so a 64-rep timing build overlaps rep r+1's projections with rep r's tail.
"""

import numpy as np
import ml_dtypes

try:
    import concourse.bass as bass
except ImportError:  # pragma: no cover
    import sys

    sys.path.insert(0, "/opt/trn_rl_repo")
    import concourse.bass as bass
from concourse import bacc

import concourse.mybir as mybir
import concourse.tile as tile
from concourse.bass_utils import run_bass_kernel_spmd

BF16 = mybir.dt.bfloat16
FP8 = mybir.dt.float8e4
F32 = mybir.dt.float32
AF = mybir.ActivationFunctionType
OP = mybir.AluOpType
PM = mybir.MatmulPerfMode

B, H, W, C = 64, 16, 16, 512
NH, D = 16, 32
S = H * W            # 256
NCORES = 8
BPC = B // NCORES    # 8 batches per core
TOK = BPC * S        # 2048 tokens per core
SCALE = D ** -0.5
AQ = 256.0           # host scale on q weights (fp8 path)
AK = 64.0            # host scale on k weights (fp8 path)
GPS_COLS = 2048      # bias-mult columns offloaded to gpsimd (of NH*S=4096)


def build_program(reps: int = 1, sections=('qkv', 'attn', 'sums', 'av', 'out'),
                  with_qkbias: bool = False, use_fp8: bool = True):
    nc = bacc.Bacc()
    xT_d = nc.dram_tensor("xT", [128, 4 * TOK], BF16, kind="ExternalInput")
    wvo_d = nc.dram_tensor("wvo", [128, 8 * 512], BF16, kind="ExternalInput")
    expb_d = nc.dram_tensor("expb", [128, 2 * NH * S], BF16, kind="ExternalInput")
    qb_d = nc.dram_tensor("qb", [1, 512], BF16, kind="ExternalInput")
    kb_d = nc.dram_tensor("kb", [1, 512], BF16, kind="ExternalInput")
    ones_r_d = nc.dram_tensor("ones_r", [1, 512], BF16, kind="ExternalInput")
    ones_c_d = nc.dram_tensor("ones_c", [128, 32], BF16, kind="ExternalInput")
    if use_fp8:
        x8_d = nc.dram_tensor("x8", [128, 4 * TOK], FP8, kind="ExternalInput")
        w8_d = nc.dram_tensor("w8", [128, 4096], FP8, kind="ExternalInput")
    else:
        wqk_d = nc.dram_tensor("wqk", [128, 8 * 512], BF16, kind="ExternalInput")
    out_d = nc.dram_tensor("out", [TOK, 512], F32, kind="ExternalOutput")

    ESCALE = float(1.0 / (AQ * AK)) if use_fp8 else 1.0

    with tile.TileContext(nc) as tc:
        import contextlib

        with contextlib.ExitStack() as ctx:
            wpool = ctx.enter_context(tc.tile_pool(name="wpool", bufs=1))
            xpool = ctx.enter_context(tc.tile_pool(name="xpool", bufs=1))
            qkpool = ctx.enter_context(tc.tile_pool(name="qkpool", bufs=1))
            epool = ctx.enter_context(tc.tile_pool(name="epool", bufs=3))
            rpool = ctx.enter_context(tc.tile_pool(name="rpool", bufs=2))
            otpool = ctx.enter_context(tc.tile_pool(name="otpool", bufs=8))
            fpool = ctx.enter_context(tc.tile_pool(name="fpool", bufs=3))
            pl_pool = ctx.enter_context(
                tc.tile_pool(name="pl", bufs=2, space="PSUM"))
            pa_pool = ctx.enter_context(
                tc.tile_pool(name="pa", bufs=1, space="PSUM"))
            ps_pool = ctx.enter_context(
                tc.tile_pool(name="ps", bufs=2, space="PSUM"))

            # ---- persistent constants (parallel DMA queues) ----
            wvo = wpool.tile([128, 8 * 512], BF16, name="wvo", tag="wvo")
            nc.scalar.dma_start(wvo[:], wvo_d[:])
            wv = [wvo[:, i * 512:(i + 1) * 512] for i in range(4)]
            wo = [wvo[:, (4 + i) * 512:(5 + i) * 512] for i in range(4)]
            if use_fp8:
                w8 = wpool.tile([128, 4096], FP8, name="w8", tag="w8")
                nc.scalar.dma_start(w8[:], w8_d[:])
                w8v = w8.rearrange("p (pj k two m) -> p pj k two m",
                                   pj=2, k=2, two=2)
            else:
                wqk = wpool.tile([128, 8 * 512], BF16, name="wqk", tag="wqk")
                nc.scalar.dma_start(wqk[:], wqk_d[:])
                wq = [wqk[:, i * 512:(i + 1) * 512] for i in range(4)]
                wk = [wqk[:, (4 + i) * 512:(5 + i) * 512] for i in range(4)]
            expb_all = wpool.tile([128, 2 * NH * S], BF16, name="expb", tag="expb")
            nc.gpsimd.dma_start(expb_all[:], expb_d[:])
            expb = [expb_all[:, t * NH * S:(t + 1) * NH * S] for t in range(2)]
            qb = wpool.tile([1, 512], BF16, name="qb", tag="qb")
            kb = wpool.tile([1, 512], BF16, name="kb", tag="kb")
            ones_r = wpool.tile([1, 512], BF16, name="ones_r", tag="ones_r")
            ones_c = wpool.tile([128, 32], BF16, name="ones_c", tag="ones_c")
            nc.gpsimd.dma_start(qb[:], qb_d[:])
            nc.gpsimd.dma_start(kb[:], kb_d[:])
            nc.gpsimd.dma_start(ones_r[:], ones_r_d[:])
            nc.gpsimd.dma_start(ones_c[:], ones_c_d[:])

            # x, chunk-major: per token-chunk tile [128, 4*512] (c-chunk, tok)
            xTn = []
            for nch in range(4):
                t_ = xpool.tile([128, 2048], BF16, name=f"xT{nch}", tag=f"xT{nch}")
                eng = [nc.sync, nc.scalar, nc.sync, nc.scalar][nch]
                eng.dma_start(t_[:], xT_d[:, nch * 2048:(nch + 1) * 2048])
                xTn.append(t_)
            if use_fp8:
                x8n = []
                for nch in range(4):
                    t_ = xpool.tile([128, 2048], FP8, name=f"x8{nch}",
                                    tag=f"x8{nch}")
                    eng = [nc.gpsimd, nc.sync, nc.gpsimd, nc.sync][nch]
                    eng.dma_start(t_[:], x8_d[:, nch * 2048:(nch + 1) * 2048])
                    x8n.append(t_)

            for _rep in range(reps):
                do = lambda s: s in sections
                # per-chunk q/k tiles: qT[nch][m], kT[nch][m] are [128, 512]
                qT = [[qkpool.tile([128, 512], BF16, name=f"qT{c}_{m}",
                                   tag=f"qT{c}_{m}") for m in range(4)]
                      for c in range(4)]
                kT = [[qkpool.tile([128, 512], BF16, name=f"kT{c}_{m}",
                                   tag=f"kT{c}_{m}") for m in range(4)]
                      for c in range(4)]
                v_sb = [qkpool.tile([128, 512], BF16, name=f"v{s}", tag=f"v{s}")
                        for s in range(TOK // 128)]

                def emit_qk_group(nch, m):
                    """q,k projections for (token chunk nch, head m-block)."""
                    for pj, dst, bt in ((0, qT, qb), (1, kT, kb)):
                        ps = ps_pool.tile([128, 512], F32, name="ps", tag="ps")
                        if use_fp8:
                            x8v = x8n[nch].rearrange(
                                "p (k two t) -> p k two t", k=2, two=2)
                            for kcs in range(2):
                                nc.tensor.matmul(
                                    ps[:, :512],
                                    w8v[:, pj, kcs, :, m * 128:(m + 1) * 128],
                                    x8v[:, kcs, :, :],
                                    start=(kcs == 0),
                                    stop=(kcs == 1 and not with_qkbias),
                                    perf_mode=PM.DoubleRow)
                        else:
                            wt = wq if pj == 0 else wk
                            for kc in range(4):
                                nc.tensor.matmul(
                                    ps[:, :512],
                                    wt[kc][:, m * 128:(m + 1) * 128],
                                    xTn[nch][:, kc * 512:(kc + 1) * 512],
                                    start=(kc == 0),
                                    stop=(kc == 3 and not with_qkbias))
                        if with_qkbias:
                            nc.tensor.matmul(
                                ps[:, :512],
                                bt[0:1, m * 128:(m + 1) * 128],
                                ones_r[0:1, :512],
                                start=False, stop=True)
                        nc.vector.tensor_copy(dst[nch][m][:], ps[:, :512])

                def emit_v_group(nch, half):
                    """v projection for 2 of the 4 s-chunks of token chunk nch."""
                    for sch in range(nch * 4 + 2 * half, nch * 4 + 2 * half + 2):
                        ps = ps_pool.tile([128, 512], F32, name="ps", tag="ps")
                        sl = sch * 128 - nch * 512
                        for kc in range(4):
                            nc.tensor.matmul(
                                ps[:, :512],
                                xTn[nch][:, kc * 512 + sl:kc * 512 + sl + 128],
                                wv[kc][:, :512],
                                start=(kc == 0), stop=(kc == 3))
                        nc.vector.tensor_copy(v_sb[sch][:], ps[:, :512])

                def emit_qkv_chunk(nch):
                    for m in range(4):
                        emit_qk_group(nch, m)
                    emit_v_group(nch, 0)
                    emit_v_group(nch, 1)

                # ---- attention, software-pipelined over batches ----
                def stage_front(b, feeder=None):
                    """logits -> exp -> bias-mul; returns E tiles for batch b.
                    feeder() emits a slice of the next QKV chunk between head
                    groups so projection PE work spreads under the ACT chain."""
                    nch_b = b // 2
                    E = []
                    for tch in range(2):
                        e0 = epool.tile([128, NH * S], BF16, name="e0", tag="e0",
                                        bufs=2)
                        toff = (b % 2) * 256 + tch * 128
                        soff = (b % 2) * 256
                        for hg in range(4):
                            for hp in range(2):
                                pl = pl_pool.tile([128, 1024], F32, name="pl",
                                                  tag="pl")
                                for hi in range(2):
                                    hl = 2 * hp + hi
                                    nc.tensor.matmul(
                                        pl[:, hi * 512:hi * 512 + 256],
                                        kT[nch_b][hg][32 * hl:32 * hl + 32,
                                                      toff:toff + 128],
                                        qT[nch_b][hg][32 * hl:32 * hl + 32,
                                                      soff:soff + 256],
                                        start=True, stop=True,
                                        tile_position=(32 * hl, 0))
                                pl_v = pl.rearrange(
                                    "p (h x) -> p h x", h=2)[:, :, :256]
                                n0 = 4 * hg + 2 * hp
                                e0_v = e0[:, n0 * 256:(n0 + 2) * 256].rearrange(
                                    "p (h x) -> p h x", h=2)
                                nc.scalar.activation(e0_v, pl_v, AF.Exp,
                                                     scale=ESCALE)
                            if feeder is not None:
                                feeder()
                        e = epool.tile([128, NH * S], BF16, name="e", tag="e",
                                       bufs=4)
                        ncol = NH * S - GPS_COLS
                        nc.vector.tensor_tensor(
                            e[:, :ncol], e0[:, :ncol], expb[tch][:, :ncol],
                            OP.mult)
                        nc.gpsimd.tensor_tensor(
                            e[:, ncol:], e0[:, ncol:], expb[tch][:, ncol:],
                            OP.mult)
                        E.append(e)
                    return E

                def stage_back(b, E):
                    """sums -> recip -> AV -> norm -> outproj -> DMA for batch b."""
                    if not do('sums'):
                        return
                    r = rpool.tile([128, 1024], F32, name="r", tag="r")
                    for sh in range(2):
                        psum_s = ps_pool.tile([128, 512], F32, name="ps", tag="ps")
                        for hg in (2 * sh, 2 * sh + 1):
                            for j in range(4):
                                n = 4 * hg + j
                                for tch in range(2):
                                    nc.tensor.matmul(
                                        psum_s[32 * j:32 * j + 32,
                                               (hg - 2 * sh) * 256:
                                               (hg - 2 * sh + 1) * 256],
                                        ones_c[:, :32],
                                        E[tch][:, n * 256:(n + 1) * 256],
                                        start=(tch == 0), stop=(tch == 1),
                                        tile_position=(0, 32 * j))
                        nc.vector.reciprocal_approx_fast(
                            r[:, sh * 512:(sh + 1) * 512], psum_s[:])
                    if not do('av'):
                        return
                    pa = pa_pool.tile([128, 1024], F32, name="pa", tag="pa")
                    for hg in range(4):
                        for j in range(4):
                            n = 4 * hg + j
                            for tch in range(2):
                                nc.tensor.matmul(
                                    pa[32 * j:32 * j + 32,
                                       hg * 256:(hg + 1) * 256],
                                    v_sb[2 * b + tch][:, n * 32:(n + 1) * 32],
                                    E[tch][:, n * 256:(n + 1) * 256],
                                    start=(tch == 0), stop=(tch == 1),
                                    tile_position=(0, 32 * j))
                    ot = otpool.tile([128, 1024], BF16, name="ot", tag="ot")
                    nc.vector.tensor_tensor(ot[:], pa[:], r[:], OP.mult)
                    if not do('out'):
                        return
                    po = pa_pool.tile([128, 1024], F32, name="po", tag="pa")
                    for sch in range(2):
                        for hg in range(4):
                            nc.tensor.matmul(
                                po[:, sch * 512:(sch + 1) * 512],
                                ot[:, hg * 256 + sch * 128:
                                   hg * 256 + (sch + 1) * 128],
                                wo[hg][:, :512],
                                start=(hg == 0), stop=(hg == 3))
                    fs = fpool.tile([128, 1024], F32, name="f", tag="f")
                    nc.scalar.copy(fs[:], po[:])
                    dst = out_d[b * S:(b + 1) * S, :].rearrange(
                        "(c p) w -> p c w", p=128)
                    nc.sync.dma_start(dst, fs.rearrange("p (c w) -> p c w", c=2))

                emit_qkv_chunk(0)
                if do('attn'):
                    # feeder: spread next-chunk QKV emission under the ACT chain
                    feed_plan = []
                    for b in range(BPC):
                        plan = []
                        nxt = b // 2 + 1
                        if b % 2 == 0 and nxt < 4:
                            plan = [lambda n=nxt, m=m: emit_qk_group(n, m)
                                    for m in range(4)]
                        elif b % 2 == 1 and nxt - 1 + 1 < 4:
                            plan = [lambda n=nxt: emit_v_group(n, 0),
                                    lambda n=nxt: emit_v_group(n, 1)]
                        feed_plan.append(plan)

                    prev = None
                    for b in range(BPC):
                        plan = list(feed_plan[b])
                        it = iter(plan)

                        def feeder(it=it):
                            nxt = next(it, None)
                            if nxt is not None:
                                nxt()
                        E = stage_front(b, feeder)
                        for fn in it:
                            fn()
                        if prev is not None:
                            stage_back(prev[0], prev[1])
                        prev = (b, E)
                    stage_back(prev[0], prev[1])
                else:
                    for nch in range(1, 4):
                        emit_qkv_chunk(nch)

    nc.compile()
    return nc


def _bias_tables(rel_emb):
    """expb[tch, t_local, n*256+s] = exp(bias[n, s, t]) with t = tch*128+t_local."""
    idx = np.arange(H)
    rel = idx[None, :] - idx[:, None] + (H - 1)
    biasT = rel_emb[:, rel.T[:, None, :, None], rel.T[None, :, None, :]]
    biasT = biasT.reshape(NH, S, S)                       # [n, t, s]
    expb = np.exp(biasT.astype(np.float64)).astype(np.float32)
    expb = np.ascontiguousarray(np.transpose(expb, (1, 0, 2)))  # [t, n, s]
    expb = expb.reshape(2, 128, NH * S).transpose(1, 0, 2).reshape(128, 2 * NH * S)
    return np.ascontiguousarray(expb).astype(ml_dtypes.bfloat16)


_CACHE = {}


def _get_program(reps=1, with_qkbias=False, use_fp8=True):
    k = (reps, with_qkbias, use_fp8)
    if k not in _CACHE:
        _CACHE[k] = build_program(reps, with_qkbias=with_qkbias,
                                  use_fp8=use_fp8)
    return _CACHE[k]


def make_in_maps(use_fp8=True, **inputs):
    x = np.asarray(inputs["x"], np.float32)
    q_w = np.asarray(inputs["q_w"], np.float32).reshape(C, NH * D)
    k_w = np.asarray(inputs["k_w"], np.float32).reshape(C, NH * D)
    v_w = np.asarray(inputs["v_w"], np.float32).reshape(C, NH * D)
    o_w = np.asarray(inputs["o_w"], np.float32).reshape(NH * D, C)
    q_b = np.asarray(inputs["q_b"], np.float32).reshape(NH * D)
    k_b = np.asarray(inputs["k_b"], np.float32).reshape(NH * D)
    rel_emb = np.asarray(inputs["rel_emb"], np.float32)

    bf = ml_dtypes.bfloat16
    f8 = ml_dtypes.float8_e4m3
    wv_s = v_w.reshape(4, 128, 512)
    wo_s = o_w.reshape(4, 128, 512)
    wvo = np.ascontiguousarray(
        np.concatenate([wv_s[i] for i in range(4)] +
                       [wo_s[i] for i in range(4)], axis=1)).astype(bf)
    expb = _bias_tables(rel_emb)

    if use_fp8:
        # w8[p, (pj, kcs, i, nd)] = w'[c = kcs*256 + i*128 + p, nd]
        wq8 = (q_w * (SCALE * AQ)).reshape(2, 2, 128, 512)   # [kcs, i, p, nd]
        wk8 = (k_w * AK).reshape(2, 2, 128, 512)
        w8 = np.stack([wq8, wk8], axis=0)                    # [pj, kcs, i, p, nd]
        w8 = np.ascontiguousarray(w8.transpose(3, 0, 1, 2, 4))  # p,pj,kcs,i,nd
        w8 = w8.reshape(128, 4096).astype(f8)
        qb_h = (q_b * (SCALE * AQ)).reshape(1, 512).astype(bf)
        kb_h = (k_b * AK).reshape(1, 512).astype(bf)
    else:
        wq_s = (q_w * SCALE).reshape(4, 128, 512)
        wk_s = k_w.reshape(4, 128, 512)
        wqk = np.ascontiguousarray(
            np.concatenate([wq_s[i] for i in range(4)] +
                           [wk_s[i] for i in range(4)], axis=1)).astype(bf)
        qb_h = (q_b * SCALE).reshape(1, 512).astype(bf)
        kb_h = k_b.reshape(1, 512).astype(bf)

    ones_r = np.ones((1, 512), bf)
    ones_c = np.ones((128, 32), bf)

    in_maps = []
    for ci in range(NCORES):
        xc = x[ci * BPC:(ci + 1) * BPC].reshape(TOK, C)     # [tok, c]
        xT4 = xc.T.reshape(4, 128, 4, 512)                  # [kc, p, nch, t]
        xT = np.ascontiguousarray(xT4.transpose(1, 2, 0, 3)  # p, nch, kc, t
                                  ).reshape(128, 4 * TOK).astype(bf)
        m = dict(xT=xT, wvo=wvo, expb=expb,
                 qb=qb_h, kb=kb_h, ones_r=ones_r, ones_c=ones_c)
        if use_fp8:
            # x8[p, (nch, kcs, i, t)] = x[c = kcs*256 + i*128 + p, tok]
            x8 = xc.T.reshape(2, 2, 128, 4, 512)            # [kcs, i, p, nch, t]
            x8 = np.ascontiguousarray(x8.transpose(2, 3, 0, 1, 4))
            m["x8"] = x8.reshape(128, 4 * TOK).astype(f8)
            m["w8"] = w8
        else:
            m["wqk"] = wqk
        in_maps.append(m)
    return in_maps


USE_FP8 = True


def kernel(**inputs):
    q_b = np.asarray(inputs["q_b"], np.float32).reshape(NH * D)
    k_b = np.asarray(inputs["k_b"], np.float32).reshape(NH * D)
    v_b = np.asarray(inputs["v_b"], np.float32).reshape(NH * D)
    o_b = np.asarray(inputs["o_b"], np.float32).reshape(C)
    o_w = np.asarray(inputs["o_w"], np.float32).reshape(NH * D, C)
    with_qkbias = bool(np.any(q_b) or np.any(k_b))
    nc = _get_program(1, with_qkbias, USE_FP8)
    in_maps = make_in_maps(use_fp8=USE_FP8, **inputs)
    res = run_bass_kernel_spmd(nc, in_maps, core_ids=list(range(NCORES)))
    outs = [res.results[ci]["out"].reshape(BPC, S, C) for ci in range(NCORES)]
    out = np.concatenate(outs, axis=0).astype(np.float32)
    # v_b rides through attention as a constant (rows of attn sum to 1); o_b is affine
    const = (v_b @ o_w) + o_b
    if np.any(const):
        out = out + const[None, None, :]
    return out


# revision 6
# speedup vs baseline: 1.0598x; 1.0598x over previous
"""Trainium2 Bass kernel for 2D-relative-bias multi-head attention.

Shapes (hardcoded): x [64, 16, 16, 512], 16 heads x 32 dim, S = 256.
Sharding: data-parallel over batch, 8 batches per core on 8 cores.

Per-core device pipeline (fp32 PSUM accumulation everywhere):
  qT/kT = W^T @ x^T            [nd, tok]   (PE; fp8 DoubleRow K=256 x2, or bf16)
  v     = x @ Wv               [tok, nd]   (PE, bf16)
  logitsT[t,s] per head        (PE, K=32, 4-head row-packed via tile_position)
  E0 = exp(scale*logitsT)      (ACT, PSUM->SBUF bf16) -- the HW critical chain
  E  = E0 * exp(biasT)         (DVE + GPSIMD column-split, bias table from host)
  sums = 1^T E (replicated)    (PE, 4-head col-packed, all-ones lhsT)
  out_unT = V^T E              (PE, 4-head col-packed)
  R = 1/sums                   (DVE reciprocal_approx_fast)
  outT = out_unT * R           (DVE)
  final = outT^T @ Wo + o_b    (PE, bf16)

q/k projections, per-token-chunk tiles: chunk c feeds batches 2c, 2c+1 only,
so a 64-rep timing build overlaps rep r+1's projections with rep r's tail.
"""

import numpy as np
import ml_dtypes

try:
    import concourse.bass as bass
except ImportError:  # pragma: no cover
    import sys

    sys.path.insert(0, "/opt/trn_rl_repo")
    import concourse.bass as bass
from concourse import bacc

import concourse.mybir as mybir
import concourse.tile as tile
from concourse.bass_utils import run_bass_kernel_spmd

BF16 = mybir.dt.bfloat16
FP8 = mybir.dt.float8e4
F32 = mybir.dt.float32
AF = mybir.ActivationFunctionType
OP = mybir.AluOpType
PM = mybir.MatmulPerfMode

B, H, W, C = 64, 16, 16, 512
NH, D = 16, 32
S = H * W            # 256
NCORES = 8
BPC = B // NCORES    # 8 batches per core
TOK = BPC * S        # 2048 tokens per core
SCALE = D ** -0.5
AQ = 256.0           # host scale on q weights (fp8 path)
AK = 64.0            # host scale on k weights (fp8 path)
GPS_COLS = 1024      # bias-mult columns offloaded to gpsimd (of NH*S=4096)


def build_program(reps: int = 1, sections=('qkv', 'attn', 'sums', 'av', 'out'),
                  with_qkbias: bool = False, use_fp8: bool = True):
    nc = bacc.Bacc()
    xT_d = nc.dram_tensor("xT", [128, 4 * TOK], BF16, kind="ExternalInput")
    wvo_d = nc.dram_tensor("wvo", [128, 8 * 512], BF16, kind="ExternalInput")
    expb_d = nc.dram_tensor("expb", [128, 2 * NH * S], BF16, kind="ExternalInput")
    qb_d = nc.dram_tensor("qb", [1, 512], BF16, kind="ExternalInput")
    kb_d = nc.dram_tensor("kb", [1, 512], BF16, kind="ExternalInput")
    ones_r_d = nc.dram_tensor("ones_r", [1, 512], BF16, kind="ExternalInput")
    ones_c_d = nc.dram_tensor("ones_c", [128, 32], BF16, kind="ExternalInput")
    if use_fp8:
        x8_d = nc.dram_tensor("x8", [128, 4 * TOK], FP8, kind="ExternalInput")
        w8_d = nc.dram_tensor("w8", [128, 4096], FP8, kind="ExternalInput")
    else:
        wqk_d = nc.dram_tensor("wqk", [128, 8 * 512], BF16, kind="ExternalInput")
    out_d = nc.dram_tensor("out", [TOK, 512], F32, kind="ExternalOutput")

    ESCALE = float(1.0 / (AQ * AK)) if use_fp8 else 1.0

    with tile.TileContext(nc) as tc:
        import contextlib

        with contextlib.ExitStack() as ctx:
            wpool = ctx.enter_context(tc.tile_pool(name="wpool", bufs=1))
            xpool = ctx.enter_context(tc.tile_pool(name="xpool", bufs=1))
            qkpool = ctx.enter_context(tc.tile_pool(name="qkpool", bufs=1))
            epool = ctx.enter_context(tc.tile_pool(name="epool", bufs=3))
            rpool = ctx.enter_context(tc.tile_pool(name="rpool", bufs=2))
            otpool = ctx.enter_context(tc.tile_pool(name="otpool", bufs=8))
            fpool = ctx.enter_context(tc.tile_pool(name="fpool", bufs=3))
            pl_pool = ctx.enter_context(
                tc.tile_pool(name="pl", bufs=2, space="PSUM"))
            pa_pool = ctx.enter_context(
                tc.tile_pool(name="pa", bufs=1, space="PSUM"))
            ps_pool = ctx.enter_context(
                tc.tile_pool(name="ps", bufs=2, space="PSUM"))

            # ---- persistent constants (parallel DMA queues) ----
            wvo = wpool.tile([128, 8 * 512], BF16, name="wvo", tag="wvo")
            nc.scalar.dma_start(wvo[:], wvo_d[:])
            wv = [wvo[:, i * 512:(i + 1) * 512] for i in range(4)]
            wo = [wvo[:, (4 + i) * 512:(5 + i) * 512] for i in range(4)]
            if use_fp8:
                w8 = wpool.tile([128, 4096], FP8, name="w8", tag="w8")
                nc.scalar.dma_start(w8[:], w8_d[:])
                w8v = w8.rearrange("p (pj k two m) -> p pj k two m",
                                   pj=2, k=2, two=2)
            else:
                wqk = wpool.tile([128, 8 * 512], BF16, name="wqk", tag="wqk")
                nc.scalar.dma_start(wqk[:], wqk_d[:])
                wq = [wqk[:, i * 512:(i + 1) * 512] for i in range(4)]
                wk = [wqk[:, (4 + i) * 512:(5 + i) * 512] for i in range(4)]
            expb_all = wpool.tile([128, 2 * NH * S], BF16, name="expb", tag="expb")
            nc.gpsimd.dma_start(expb_all[:], expb_d[:])
            expb = [expb_all[:, t * NH * S:(t + 1) * NH * S] for t in range(2)]
            qb = wpool.tile([1, 512], BF16, name="qb", tag="qb")
            kb = wpool.tile([1, 512], BF16, name="kb", tag="kb")
            ones_r = wpool.tile([1, 512], BF16, name="ones_r", tag="ones_r")
            ones_c = wpool.tile([128, 32], BF16, name="ones_c", tag="ones_c")
            nc.gpsimd.dma_start(qb[:], qb_d[:])
            nc.gpsimd.dma_start(kb[:], kb_d[:])
            nc.gpsimd.dma_start(ones_r[:], ones_r_d[:])
            nc.gpsimd.dma_start(ones_c[:], ones_c_d[:])

            # x, chunk-major: per token-chunk tile [128, 4*512] (c-chunk, tok)
            xTn = []
            for nch in range(4):
                t_ = xpool.tile([128, 2048], BF16, name=f"xT{nch}", tag=f"xT{nch}")
                eng = [nc.sync, nc.scalar, nc.sync, nc.scalar][nch]
                eng.dma_start(t_[:], xT_d[:, nch * 2048:(nch + 1) * 2048])
                xTn.append(t_)
            if use_fp8:
                x8n = []
                for nch in range(4):
                    t_ = xpool.tile([128, 2048], FP8, name=f"x8{nch}",
                                    tag=f"x8{nch}")
                    eng = [nc.gpsimd, nc.sync, nc.gpsimd, nc.sync][nch]
                    eng.dma_start(t_[:], x8_d[:, nch * 2048:(nch + 1) * 2048])
                    x8n.append(t_)

            for _rep in range(reps):
                do = lambda s: s in sections
                # per-chunk q/k tiles: qT[nch][m], kT[nch][m] are [128, 512]
                qT = [[qkpool.tile([128, 512], BF16, name=f"qT{c}_{m}",
                                   tag=f"qT{c}_{m}") for m in range(4)]
                      for c in range(4)]
                kT = [[qkpool.tile([128, 512], BF16, name=f"kT{c}_{m}",
                                   tag=f"kT{c}_{m}") for m in range(4)]
                      for c in range(4)]
                v_sb = [qkpool.tile([128, 512], BF16, name=f"v{s}", tag=f"v{s}")
                        for s in range(TOK // 128)]

                def emit_qk_group(nch, m):
                    """q,k projections for (token chunk nch, head m-block)."""
                    for pj, dst, bt in ((0, qT, qb), (1, kT, kb)):
                        ps = ps_pool.tile([128, 512], F32, name="ps", tag="ps")
                        if use_fp8:
                            x8v = x8n[nch].rearrange(
                                "p (k two t) -> p k two t", k=2, two=2)
                            for kcs in range(2):
                                nc.tensor.matmul(
                                    ps[:, :512],
                                    w8v[:, pj, kcs, :, m * 128:(m + 1) * 128],
                                    x8v[:, kcs, :, :],
                                    start=(kcs == 0),
                                    stop=(kcs == 1 and not with_qkbias),
                                    perf_mode=PM.DoubleRow)
                        else:
                            wt = wq if pj == 0 else wk
                            for kc in range(4):
                                nc.tensor.matmul(
                                    ps[:, :512],
                                    wt[kc][:, m * 128:(m + 1) * 128],
                                    xTn[nch][:, kc * 512:(kc + 1) * 512],
                                    start=(kc == 0),
                                    stop=(kc == 3 and not with_qkbias))
                        if with_qkbias:
                            nc.tensor.matmul(
                                ps[:, :512],
                                bt[0:1, m * 128:(m + 1) * 128],
                                ones_r[0:1, :512],
                                start=False, stop=True)
                        nc.vector.tensor_copy(dst[nch][m][:], ps[:, :512])

                def emit_v_group(nch, half):
                    """v projection for 2 of the 4 s-chunks of token chunk nch."""
                    for sch in range(nch * 4 + 2 * half, nch * 4 + 2 * half + 2):
                        ps = ps_pool.tile([128, 512], F32, name="ps", tag="ps")
                        sl = sch * 128 - nch * 512
                        for kc in range(4):
                            nc.tensor.matmul(
                                ps[:, :512],
                                xTn[nch][:, kc * 512 + sl:kc * 512 + sl + 128],
                                wv[kc][:, :512],
                                start=(kc == 0), stop=(kc == 3))
                        nc.vector.tensor_copy(v_sb[sch][:], ps[:, :512])

                def emit_qkv_chunk(nch):
                    for m in range(4):
                        emit_qk_group(nch, m)
                    emit_v_group(nch, 0)
                    emit_v_group(nch, 1)

                # ---- attention, software-pipelined over batches ----
                def stage_front(b, feeder=None):
                    """logits -> exp -> bias-mul; returns E tiles for batch b.
                    feeder() emits a slice of the next QKV chunk between head
                    groups so projection PE work spreads under the ACT chain."""
                    nch_b = b // 2
                    E = []
                    for tch in range(2):
                        e0 = epool.tile([128, NH * S], BF16, name="e0", tag="e0",
                                        bufs=2)
                        toff = (b % 2) * 256 + tch * 128
                        soff = (b % 2) * 256
                        for hg in range(4):
                            for hp in range(2):
                                pl = pl_pool.tile([128, 1024], F32, name="pl",
                                                  tag="pl")
                                for hi in range(2):
                                    hl = 2 * hp + hi
                                    nc.tensor.matmul(
                                        pl[:, hi * 512:hi * 512 + 256],
                                        kT[nch_b][hg][32 * hl:32 * hl + 32,
                                                      toff:toff + 128],
                                        qT[nch_b][hg][32 * hl:32 * hl + 32,
                                                      soff:soff + 256],
                                        start=True, stop=True,
                                        tile_position=(32 * hl, 0))
                                pl_v = pl.rearrange(
                                    "p (h x) -> p h x", h=2)[:, :, :256]
                                n0 = 4 * hg + 2 * hp
                                e0_v = e0[:, n0 * 256:(n0 + 2) * 256].rearrange(
                                    "p (h x) -> p h x", h=2)
                                nc.scalar.activation(e0_v, pl_v, AF.Exp,
                                                     scale=ESCALE)
                            if feeder is not None:
                                feeder()
                        e = epool.tile([128, NH * S], BF16, name="e", tag="e",
                                       bufs=4)
                        ncol = NH * S - GPS_COLS
                        nc.vector.tensor_tensor(
                            e[:, :ncol], e0[:, :ncol], expb[tch][:, :ncol],
                            OP.mult)
                        nc.gpsimd.tensor_tensor(
                            e[:, ncol:], e0[:, ncol:], expb[tch][:, ncol:],
                            OP.mult)
                        E.append(e)
                    return E

                def stage_back(b, E):
                    """sums -> recip -> AV -> norm -> outproj -> DMA for batch b."""
                    if not do('sums'):
                        return
                    r = rpool.tile([128, 1024], F32, name="r", tag="r")
                    for sh in range(2):
                        psum_s = ps_pool.tile([128, 512], F32, name="ps", tag="ps")
                        for hg in (2 * sh, 2 * sh + 1):
                            for j in range(4):
                                n = 4 * hg + j
                                for tch in range(2):
                                    nc.tensor.matmul(
                                        psum_s[32 * j:32 * j + 32,
                                               (hg - 2 * sh) * 256:
                                               (hg - 2 * sh + 1) * 256],
                                        ones_c[:, :32],
                                        E[tch][:, n * 256:(n + 1) * 256],
                                        start=(tch == 0), stop=(tch == 1),
                                        tile_position=(0, 32 * j))
                        nc.vector.reciprocal_approx_fast(
                            r[:, sh * 512:(sh + 1) * 512], psum_s[:])
                    if not do('av'):
                        return
                    pa = pa_pool.tile([128, 1024], F32, name="pa", tag="pa")
                    for hg in range(4):
                        for j in range(4):
                            n = 4 * hg + j
                            for tch in range(2):
                                nc.tensor.matmul(
                                    pa[32 * j:32 * j + 32,
                                       hg * 256:(hg + 1) * 256],
                                    v_sb[2 * b + tch][:, n * 32:(n + 1) * 32],
                                    E[tch][:, n * 256:(n + 1) * 256],
                                    start=(tch == 0), stop=(tch == 1),
                                    tile_position=(0, 32 * j))
                    ot = otpool.tile([128, 1024], BF16, name="ot", tag="ot")
                    nc.vector.tensor_tensor(ot[:], pa[:], r[:], OP.mult)
                    if not do('out'):
                        return
                    po = pa_pool.tile([128, 1024], F32, name="po", tag="pa")
                    for sch in range(2):
                        for hg in range(4):
                            nc.tensor.matmul(
                                po[:, sch * 512:(sch + 1) * 512],
                                ot[:, hg * 256 + sch * 128:
                                   hg * 256 + (sch + 1) * 128],
                                wo[hg][:, :512],
                                start=(hg == 0), stop=(hg == 3))
                    fs = fpool.tile([128, 1024], F32, name="f", tag="f")
                    nc.scalar.copy(fs[:], po[:])
                    dst = out_d[b * S:(b + 1) * S, :].rearrange(
                        "(c p) w -> p c w", p=128)
                    nc.sync.dma_start(dst, fs.rearrange("p (c w) -> p c w", c=2))

                emit_qkv_chunk(0)
                if do('attn'):
                    # feeder: spread next-chunk QKV emission under the ACT chain
                    feed_plan = []
                    for b in range(BPC):
                        plan = []
                        nxt = b // 2 + 1
                        if b % 2 == 0 and nxt < 4:
                            plan = [lambda n=nxt, m=m: emit_qk_group(n, m)
                                    for m in range(4)]
                        elif b % 2 == 1 and nxt - 1 + 1 < 4:
                            plan = [lambda n=nxt: emit_v_group(n, 0),
                                    lambda n=nxt: emit_v_group(n, 1)]
                        feed_plan.append(plan)

                    prev = None
                    for b in range(BPC):
                        plan = list(feed_plan[b])
                        it = iter(plan)

                        def feeder(it=it):
                            nxt = next(it, None)
                            if nxt is not None:
                                nxt()
                        E = stage_front(b, feeder)
                        for fn in it:
                            fn()
                        if prev is not None:
                            stage_back(prev[0], prev[1])
                        prev = (b, E)
                    stage_back(prev[0], prev[1])
                else:
                    for nch in range(1, 4):
                        emit_qkv_chunk(nch)

    nc.compile()
    return nc


def _bias_tables(rel_emb):
    """expb[tch, t_local, n*256+s] = exp(bias[n, s, t]) with t = tch*128+t_local."""
    idx = np.arange(H)
    rel = idx[None, :] - idx[:, None] + (H - 1)
    biasT = rel_emb[:, rel.T[:, None, :, None], rel.T[None, :, None, :]]
    biasT = biasT.reshape(NH, S, S)                       # [n, t, s]
    expb = np.exp(biasT.astype(np.float64)).astype(np.float32)
    expb = np.ascontiguousarray(np.transpose(expb, (1, 0, 2)))  # [t, n, s]
    expb = expb.reshape(2, 128, NH * S).transpose(1, 0, 2).reshape(128, 2 * NH * S)
    return np.ascontiguousarray(expb).astype(ml_dtypes.bfloat16)


_CACHE = {}


def _get_program(reps=1, with_qkbias=False, use_fp8=True):
    k = (reps, with_qkbias, use_fp8)
    if k not in _CACHE:
        _CACHE[k] = build_program(reps, with_qkbias=with_qkbias,
                                  use_fp8=use_fp8)
    return _CACHE[k]


def make_in_maps(use_fp8=True, **inputs):
    x = np.asarray(inputs["x"], np.float32)
    q_w = np.asarray(inputs["q_w"], np.float32).reshape(C, NH * D)
    k_w = np.asarray(inputs["k_w"], np.float32).reshape(C, NH * D)
    v_w = np.asarray(inputs["v_w"], np.float32).reshape(C, NH * D)
    o_w = np.asarray(inputs["o_w"], np.float32).reshape(NH * D, C)
    q_b = np.asarray(inputs["q_b"], np.float32).reshape(NH * D)
    k_b = np.asarray(inputs["k_b"], np.float32).reshape(NH * D)
    rel_emb = np.asarray(inputs["rel_emb"], np.float32)

    bf = ml_dtypes.bfloat16
    f8 = ml_dtypes.float8_e4m3
    wv_s = v_w.reshape(4, 128, 512)
    wo_s = o_w.reshape(4, 128, 512)
    wvo = np.ascontiguousarray(
        np.concatenate([wv_s[i] for i in range(4)] +
                       [wo_s[i] for i in range(4)], axis=1)).astype(bf)
    expb = _bias_tables(rel_emb)

    if use_fp8:
        # w8[p, (pj, kcs, i, nd)] = w'[c = kcs*256 + i*128 + p, nd]
        wq8 = (q_w * (SCALE * AQ)).reshape(2, 2, 128, 512)   # [kcs, i, p, nd]
        wk8 = (k_w * AK).reshape(2, 2, 128, 512)
        w8 = np.stack([wq8, wk8], axis=0)                    # [pj, kcs, i, p, nd]
        w8 = np.ascontiguousarray(w8.transpose(3, 0, 1, 2, 4))  # p,pj,kcs,i,nd
        w8 = w8.reshape(128, 4096).astype(f8)
        qb_h = (q_b * (SCALE * AQ)).reshape(1, 512).astype(bf)
        kb_h = (k_b * AK).reshape(1, 512).astype(bf)
    else:
        wq_s = (q_w * SCALE).reshape(4, 128, 512)
        wk_s = k_w.reshape(4, 128, 512)
        wqk = np.ascontiguousarray(
            np.concatenate([wq_s[i] for i in range(4)] +
                           [wk_s[i] for i in range(4)], axis=1)).astype(bf)
        qb_h = (q_b * SCALE).reshape(1, 512).astype(bf)
        kb_h = k_b.reshape(1, 512).astype(bf)

    ones_r = np.ones((1, 512), bf)
    ones_c = np.ones((128, 32), bf)

    in_maps = []
    for ci in range(NCORES):
        xc = x[ci * BPC:(ci + 1) * BPC].reshape(TOK, C)     # [tok, c]
        xT4 = xc.T.reshape(4, 128, 4, 512)                  # [kc, p, nch, t]
        xT = np.ascontiguousarray(xT4.transpose(1, 2, 0, 3)  # p, nch, kc, t
                                  ).reshape(128, 4 * TOK).astype(bf)
        m = dict(xT=xT, wvo=wvo, expb=expb,
                 qb=qb_h, kb=kb_h, ones_r=ones_r, ones_c=ones_c)
        if use_fp8:
            # x8[p, (nch, kcs, i, t)] = x[c = kcs*256 + i*128 + p, tok]
            x8 = xc.T.reshape(2, 2, 128, 4, 512)            # [kcs, i, p, nch, t]
            x8 = np.ascontiguousarray(x8.transpose(2, 3, 0, 1, 4))
            m["x8"] = x8.reshape(128, 4 * TOK).astype(f8)
            m["w8"] = w8
        else:
            m["wqk"] = wqk
        in_maps.append(m)
    return in_maps


USE_FP8 = True


def kernel(**inputs):
    q_b = np.asarray(inputs["q_b"], np.float32).reshape(NH * D)
    k_b = np.asarray(inputs["k_b"], np.float32).reshape(NH * D)
    v_b = np.asarray(inputs["v_b"], np.float32).reshape(NH * D)
    o_b = np.asarray(inputs["o_b"], np.float32).reshape(C)
    o_w = np.asarray(inputs["o_w"], np.float32).reshape(NH * D, C)
    with_qkbias = bool(np.any(q_b) or np.any(k_b))
    nc = _get_program(1, with_qkbias, USE_FP8)
    in_maps = make_in_maps(use_fp8=USE_FP8, **inputs)
    res = run_bass_kernel_spmd(nc, in_maps, core_ids=list(range(NCORES)))
    outs = [res.results[ci]["out"].reshape(BPC, S, C) for ci in range(NCORES)]
    out = np.concatenate(outs, axis=0).astype(np.float32)
    # v_b rides through attention as a constant (rows of attn sum to 1); o_b is affine
    const = (v_b @ o_w) + o_b
    if np.any(const):
        out = out + const[None, None, :]
    return out


# revision 8
# speedup vs baseline: 1.2064x; 1.1383x over previous
"""Trainium2 Bass kernel for 2D-relative-bias multi-head attention.

Shapes (hardcoded): x [64, 16, 16, 512], 16 heads x 32 dim, S = 256.
Sharding: data-parallel over batch, 8 batches per core on 8 cores.

Per-core device pipeline (fp32 PSUM accumulation everywhere):
  qT/kT = W^T @ x^T            [nd, tok]   (PE; fp8 DoubleRow K=256 x2, or bf16)
  v     = x @ Wv               [tok, nd]   (PE, bf16)
  logitsT[t,s] per head        (PE, K=32, 4-head row-packed via tile_position)
  E0 = exp(scale*logitsT)      (ACT, PSUM->SBUF bf16) -- the HW critical chain
  E  = E0 * exp(biasT)         (DVE + GPSIMD column-split, bias table from host)
  sums = 1^T E (replicated)    (PE, 4-head col-packed, all-ones lhsT)
  out_unT = V^T E              (PE, 4-head col-packed)
  R = 1/sums                   (DVE reciprocal_approx_fast)
  outT = out_unT * R           (DVE)
  final = outT^T @ Wo + o_b    (PE, bf16)

q/k projections, per-token-chunk tiles: chunk c feeds batches 2c, 2c+1 only,
so a 64-rep timing build overlaps rep r+1's projections with rep r's tail.
"""

import numpy as np
import ml_dtypes

try:
    import concourse.bass as bass
except ImportError:  # pragma: no cover
    import sys

    sys.path.insert(0, "/opt/trn_rl_repo")
    import concourse.bass as bass
from concourse import bacc

import concourse.mybir as mybir
import concourse.tile as tile
from concourse.bass_utils import run_bass_kernel_spmd

BF16 = mybir.dt.bfloat16
FP8 = mybir.dt.float8e4
F32 = mybir.dt.float32
AF = mybir.ActivationFunctionType
OP = mybir.AluOpType
PM = mybir.MatmulPerfMode

B, H, W, C = 64, 16, 16, 512
NH, D = 16, 32
S = H * W            # 256
NCORES = 8
BPC = B // NCORES    # 8 batches per core
TOK = BPC * S        # 2048 tokens per core
SCALE = D ** -0.5
AQ = 256.0           # host scale on q weights (fp8 path)
AK = 64.0            # host scale on k weights (fp8 path)
GPS_COLS = 1024      # bias-mult columns offloaded to gpsimd (of NH*S=4096)


def build_program(reps: int = 1, sections=('qkv', 'attn', 'sums', 'av', 'out'),
                  with_qkbias: bool = False, use_fp8: bool = True):
    nc = bacc.Bacc()
    xT_d = nc.dram_tensor("xT", [128, 4 * TOK], BF16, kind="ExternalInput")
    wvo_d = nc.dram_tensor("wvo", [128, 8 * 512], BF16, kind="ExternalInput")
    expb_d = nc.dram_tensor("expb", [128, 2 * NH * S], BF16, kind="ExternalInput")
    qb_d = nc.dram_tensor("qb", [1, 512], BF16, kind="ExternalInput")
    kb_d = nc.dram_tensor("kb", [1, 512], BF16, kind="ExternalInput")
    ones_r_d = nc.dram_tensor("ones_r", [1, 512], BF16, kind="ExternalInput")
    ones_c_d = nc.dram_tensor("ones_c", [128, 32], BF16, kind="ExternalInput")
    if use_fp8:
        x8_d = nc.dram_tensor("x8", [128, 4 * TOK], FP8, kind="ExternalInput")
        w8_d = nc.dram_tensor("w8", [128, 4096], FP8, kind="ExternalInput")
    else:
        wqk_d = nc.dram_tensor("wqk", [128, 8 * 512], BF16, kind="ExternalInput")
    out_d = nc.dram_tensor("out", [TOK, 512], F32, kind="ExternalOutput")

    ESCALE = float(1.0 / (AQ * AK)) if use_fp8 else 1.0

    with tile.TileContext(nc) as tc:
        import contextlib

        with contextlib.ExitStack() as ctx:
            wpool = ctx.enter_context(tc.tile_pool(name="wpool", bufs=1))
            xpool = ctx.enter_context(tc.tile_pool(name="xpool", bufs=1))
            qkpool = ctx.enter_context(tc.tile_pool(name="qkpool", bufs=1))
            epool = ctx.enter_context(tc.tile_pool(name="epool", bufs=3))
            rpool = ctx.enter_context(tc.tile_pool(name="rpool", bufs=2))
            otpool = ctx.enter_context(tc.tile_pool(name="otpool", bufs=8))
            fpool = ctx.enter_context(tc.tile_pool(name="fpool", bufs=3))
            pl_pool = ctx.enter_context(
                tc.tile_pool(name="pl", bufs=2, space="PSUM"))
            pa_pool = ctx.enter_context(
                tc.tile_pool(name="pa", bufs=1, space="PSUM"))
            ps_pool = ctx.enter_context(
                tc.tile_pool(name="ps", bufs=2, space="PSUM"))

            # ---- persistent constants (parallel DMA queues) ----
            wvo = wpool.tile([128, 8 * 512], BF16, name="wvo", tag="wvo")
            nc.scalar.dma_start(wvo[:], wvo_d[:])
            wv = [wvo[:, i * 512:(i + 1) * 512] for i in range(4)]
            wo = [wvo[:, (4 + i) * 512:(5 + i) * 512] for i in range(4)]
            if use_fp8:
                w8 = wpool.tile([128, 4096], FP8, name="w8", tag="w8")
                nc.scalar.dma_start(w8[:], w8_d[:])
                w8v = w8.rearrange("p (pj k two m) -> p pj k two m",
                                   pj=2, k=2, two=2)
            else:
                wqk = wpool.tile([128, 8 * 512], BF16, name="wqk", tag="wqk")
                nc.scalar.dma_start(wqk[:], wqk_d[:])
                wq = [wqk[:, i * 512:(i + 1) * 512] for i in range(4)]
                wk = [wqk[:, (4 + i) * 512:(5 + i) * 512] for i in range(4)]
            expb_all = wpool.tile([128, 2 * NH * S], BF16, name="expb", tag="expb")
            nc.gpsimd.dma_start(expb_all[:], expb_d[:])
            expb = [expb_all[:, t * NH * S:(t + 1) * NH * S] for t in range(2)]
            qb = wpool.tile([1, 512], BF16, name="qb", tag="qb")
            kb = wpool.tile([1, 512], BF16, name="kb", tag="kb")
            ones_r = wpool.tile([1, 512], BF16, name="ones_r", tag="ones_r")
            ones_c = wpool.tile([128, 32], BF16, name="ones_c", tag="ones_c")
            nc.gpsimd.dma_start(qb[:], qb_d[:])
            nc.gpsimd.dma_start(kb[:], kb_d[:])
            nc.gpsimd.dma_start(ones_r[:], ones_r_d[:])
            nc.gpsimd.dma_start(ones_c[:], ones_c_d[:])

            # x, chunk-major: per token-chunk tile [128, 4*512] (c-chunk, tok)
            xTn = []
            for nch in range(4):
                t_ = xpool.tile([128, 2048], BF16, name=f"xT{nch}", tag=f"xT{nch}")
                eng = [nc.sync, nc.scalar, nc.sync, nc.scalar][nch]
                eng.dma_start(t_[:], xT_d[:, nch * 2048:(nch + 1) * 2048])
                xTn.append(t_)
            if use_fp8:
                x8n = []
                for nch in range(4):
                    t_ = xpool.tile([128, 2048], FP8, name=f"x8{nch}",
                                    tag=f"x8{nch}")
                    eng = [nc.gpsimd, nc.sync, nc.gpsimd, nc.sync][nch]
                    eng.dma_start(t_[:], x8_d[:, nch * 2048:(nch + 1) * 2048])
                    x8n.append(t_)

            do = lambda s: s in sections
            # per-chunk q/k tiles: qT[nch][m], kT[nch][m] are [128, 512]
            qT = [[qkpool.tile([128, 512], BF16, name=f"qT{c}_{m}",
                               tag=f"qT{c}_{m}") for m in range(4)]
                  for c in range(4)]
            kT = [[qkpool.tile([128, 512], BF16, name=f"kT{c}_{m}",
                               tag=f"kT{c}_{m}") for m in range(4)]
                  for c in range(4)]
            v_sb = [qkpool.tile([128, 512], BF16, name=f"v{s}", tag=f"v{s}")
                    for s in range(TOK // 128)]

            for _rep in range(reps):

                def emit_qk_group(nch, m):
                    """q,k projections for (token chunk nch, head m-block)."""
                    for pj, dst, bt in ((0, qT, qb), (1, kT, kb)):
                        ps = ps_pool.tile([128, 512], F32, name="ps", tag="ps")
                        if use_fp8:
                            x8v = x8n[nch].rearrange(
                                "p (k two t) -> p k two t", k=2, two=2)
                            for kcs in range(2):
                                nc.tensor.matmul(
                                    ps[:, :512],
                                    w8v[:, pj, kcs, :, m * 128:(m + 1) * 128],
                                    x8v[:, kcs, :, :],
                                    start=(kcs == 0),
                                    stop=(kcs == 1 and not with_qkbias),
                                    perf_mode=PM.DoubleRow)
                        else:
                            wt = wq if pj == 0 else wk
                            for kc in range(4):
                                nc.tensor.matmul(
                                    ps[:, :512],
                                    wt[kc][:, m * 128:(m + 1) * 128],
                                    xTn[nch][:, kc * 512:(kc + 1) * 512],
                                    start=(kc == 0),
                                    stop=(kc == 3 and not with_qkbias))
                        if with_qkbias:
                            nc.tensor.matmul(
                                ps[:, :512],
                                bt[0:1, m * 128:(m + 1) * 128],
                                ones_r[0:1, :512],
                                start=False, stop=True)
                        nc.vector.tensor_copy(dst[nch][m][:], ps[:, :512])

                def emit_v_group(nch, half):
                    """v projection for 2 of the 4 s-chunks of token chunk nch."""
                    for sch in range(nch * 4 + 2 * half, nch * 4 + 2 * half + 2):
                        ps = ps_pool.tile([128, 512], F32, name="ps", tag="ps")
                        sl = sch * 128 - nch * 512
                        for kc in range(4):
                            nc.tensor.matmul(
                                ps[:, :512],
                                xTn[nch][:, kc * 512 + sl:kc * 512 + sl + 128],
                                wv[kc][:, :512],
                                start=(kc == 0), stop=(kc == 3))
                        nc.vector.tensor_copy(v_sb[sch][:], ps[:, :512])

                def emit_qkv_chunk(nch):
                    for m in range(4):
                        emit_qk_group(nch, m)
                    emit_v_group(nch, 0)
                    emit_v_group(nch, 1)

                # ---- attention, software-pipelined over batches ----
                def stage_front(b, feeder=None):
                    """logits -> exp -> bias-mul; returns E tiles for batch b.
                    feeder() emits a slice of the next QKV chunk between head
                    groups so projection PE work spreads under the ACT chain."""
                    nch_b = b // 2
                    E = []
                    for tch in range(2):
                        e0 = epool.tile([128, NH * S], BF16, name="e0", tag="e0",
                                        bufs=2)
                        toff = (b % 2) * 256 + tch * 128
                        soff = (b % 2) * 256
                        for hg in range(4):
                            for hp in range(2):
                                pl = pl_pool.tile([128, 1024], F32, name="pl",
                                                  tag="pl")
                                for hi in range(2):
                                    hl = 2 * hp + hi
                                    nc.tensor.matmul(
                                        pl[:, hi * 512:hi * 512 + 256],
                                        kT[nch_b][hg][32 * hl:32 * hl + 32,
                                                      toff:toff + 128],
                                        qT[nch_b][hg][32 * hl:32 * hl + 32,
                                                      soff:soff + 256],
                                        start=True, stop=True,
                                        tile_position=(32 * hl, 0))
                                pl_v = pl.rearrange(
                                    "p (h x) -> p h x", h=2)[:, :, :256]
                                n0 = 4 * hg + 2 * hp
                                e0_v = e0[:, n0 * 256:(n0 + 2) * 256].rearrange(
                                    "p (h x) -> p h x", h=2)
                                nc.scalar.activation(e0_v, pl_v, AF.Exp,
                                                     scale=ESCALE)
                            if feeder is not None:
                                feeder()
                        e = epool.tile([128, NH * S], BF16, name="e", tag="e",
                                       bufs=4)
                        ncol = NH * S - GPS_COLS
                        nc.vector.tensor_tensor(
                            e[:, :ncol], e0[:, :ncol], expb[tch][:, :ncol],
                            OP.mult)
                        nc.gpsimd.tensor_tensor(
                            e[:, ncol:], e0[:, ncol:], expb[tch][:, ncol:],
                            OP.mult)
                        E.append(e)
                    return E

                def stage_back(b, E):
                    """sums -> recip -> AV -> norm -> outproj -> DMA for batch b."""
                    if not do('sums'):
                        return
                    r = rpool.tile([128, 1024], F32, name="r", tag="r")
                    for sh in range(2):
                        psum_s = ps_pool.tile([128, 512], F32, name="ps", tag="ps")
                        for hg in (2 * sh, 2 * sh + 1):
                            for j in range(4):
                                n = 4 * hg + j
                                for tch in range(2):
                                    nc.tensor.matmul(
                                        psum_s[32 * j:32 * j + 32,
                                               (hg - 2 * sh) * 256:
                                               (hg - 2 * sh + 1) * 256],
                                        ones_c[:, :32],
                                        E[tch][:, n * 256:(n + 1) * 256],
                                        start=(tch == 0), stop=(tch == 1),
                                        tile_position=(0, 32 * j))
                        nc.vector.reciprocal_approx_fast(
                            r[:, sh * 512:(sh + 1) * 512], psum_s[:])
                    if not do('av'):
                        return
                    pa = pa_pool.tile([128, 1024], F32, name="pa", tag="pa")
                    for hg in range(4):
                        for j in range(4):
                            n = 4 * hg + j
                            for tch in range(2):
                                nc.tensor.matmul(
                                    pa[32 * j:32 * j + 32,
                                       hg * 256:(hg + 1) * 256],
                                    v_sb[2 * b + tch][:, n * 32:(n + 1) * 32],
                                    E[tch][:, n * 256:(n + 1) * 256],
                                    start=(tch == 0), stop=(tch == 1),
                                    tile_position=(0, 32 * j))
                    ot = otpool.tile([128, 1024], BF16, name="ot", tag="ot")
                    nc.vector.tensor_tensor(ot[:], pa[:], r[:], OP.mult)
                    if not do('out'):
                        return
                    po = pa_pool.tile([128, 1024], F32, name="po", tag="pa")
                    for sch in range(2):
                        for hg in range(4):
                            nc.tensor.matmul(
                                po[:, sch * 512:(sch + 1) * 512],
                                ot[:, hg * 256 + sch * 128:
                                   hg * 256 + (sch + 1) * 128],
                                wo[hg][:, :512],
                                start=(hg == 0), stop=(hg == 3))
                    fs = fpool.tile([128, 1024], F32, name="f", tag="f")
                    nc.vector.tensor_copy(fs[:], po[:])
                    dst = out_d[b * S:(b + 1) * S, :].rearrange(
                        "(c p) w -> p c w", p=128)
                    nc.sync.dma_start(dst, fs.rearrange("p (c w) -> p c w", c=2))

                if _rep == 0:
                    for m in range(4):
                        emit_qk_group(0, m)
                if do('attn'):
                    # feeder: spread QKV emission under the ACT chain.
                    # qk(c) deadline: front(2c); v(c) deadline: back(2c)@slot 2c+1.
                    # slots 6,7 pre-emit NEXT rep's chunk 0 (tiles are tag-stable).
                    last = _rep == reps - 1
                    feed_plan = [
                        [lambda: emit_v_group(0, 0), lambda: emit_v_group(0, 1),
                         lambda: emit_qk_group(1, 0), lambda: emit_qk_group(1, 1)],
                        [lambda: emit_qk_group(1, 2), lambda: emit_qk_group(1, 3),
                         lambda: emit_v_group(1, 0)],
                        [lambda: emit_v_group(1, 1),
                         lambda: emit_qk_group(2, 0), lambda: emit_qk_group(2, 1)],
                        [lambda: emit_qk_group(2, 2), lambda: emit_qk_group(2, 3),
                         lambda: emit_v_group(2, 0)],
                        [lambda: emit_v_group(2, 1),
                         lambda: emit_qk_group(3, 0), lambda: emit_qk_group(3, 1)],
                        [lambda: emit_qk_group(3, 2), lambda: emit_qk_group(3, 3),
                         lambda: emit_v_group(3, 0)],
                        [lambda: emit_v_group(3, 1)] + ([] if last else
                            [lambda: emit_qk_group(0, 0),
                             lambda: emit_qk_group(0, 1)]),
                        [] if last else
                        [lambda: emit_qk_group(0, 2), lambda: emit_qk_group(0, 3)],
                    ]
                    prev = None
                    for b in range(BPC):
                        it = iter(feed_plan[b])

                        def feeder(it=it):
                            nxt = next(it, None)
                            if nxt is not None:
                                nxt()
                        E = stage_front(b, feeder)
                        for fn in it:
                            fn()
                        if prev is not None:
                            stage_back(prev[0], prev[1])
                        prev = (b, E)
                    stage_back(prev[0], prev[1])
                else:
                    emit_v_group(0, 0)
                    emit_v_group(0, 1)
                    for nch in range(1, 4):
                        emit_qkv_chunk(nch)

    nc.compile()
    return nc


def _bias_tables(rel_emb):
    """expb[tch, t_local, n*256+s] = exp(bias[n, s, t]) with t = tch*128+t_local."""
    idx = np.arange(H)
    rel = idx[None, :] - idx[:, None] + (H - 1)
    biasT = rel_emb[:, rel.T[:, None, :, None], rel.T[None, :, None, :]]
    biasT = biasT.reshape(NH, S, S)                       # [n, t, s]
    expb = np.exp(biasT.astype(np.float64)).astype(np.float32)
    expb = np.ascontiguousarray(np.transpose(expb, (1, 0, 2)))  # [t, n, s]
    expb = expb.reshape(2, 128, NH * S).transpose(1, 0, 2).reshape(128, 2 * NH * S)
    return np.ascontiguousarray(expb).astype(ml_dtypes.bfloat16)


_CACHE = {}


def _get_program(reps=1, with_qkbias=False, use_fp8=True):
    k = (reps, with_qkbias, use_fp8)
    if k not in _CACHE:
        _CACHE[k] = build_program(reps, with_qkbias=with_qkbias,
                                  use_fp8=use_fp8)
    return _CACHE[k]


def make_in_maps(use_fp8=True, **inputs):
    x = np.asarray(inputs["x"], np.float32)
    q_w = np.asarray(inputs["q_w"], np.float32).reshape(C, NH * D)
    k_w = np.asarray(inputs["k_w"], np.float32).reshape(C, NH * D)
    v_w = np.asarray(inputs["v_w"], np.float32).reshape(C, NH * D)
    o_w = np.asarray(inputs["o_w"], np.float32).reshape(NH * D, C)
    q_b = np.asarray(inputs["q_b"], np.float32).reshape(NH * D)
    k_b = np.asarray(inputs["k_b"], np.float32).reshape(NH * D)
    rel_emb = np.asarray(inputs["rel_emb"], np.float32)

    bf = ml_dtypes.bfloat16
    f8 = ml_dtypes.float8_e4m3
    wv_s = v_w.reshape(4, 128, 512)
    wo_s = o_w.reshape(4, 128, 512)
    wvo = np.ascontiguousarray(
        np.concatenate([wv_s[i] for i in range(4)] +
                       [wo_s[i] for i in range(4)], axis=1)).astype(bf)
    expb = _bias_tables(rel_emb)

    if use_fp8:
        # w8[p, (pj, kcs, i, nd)] = w'[c = kcs*256 + i*128 + p, nd]
        wq8 = (q_w * (SCALE * AQ)).reshape(2, 2, 128, 512)   # [kcs, i, p, nd]
        wk8 = (k_w * AK).reshape(2, 2, 128, 512)
        w8 = np.stack([wq8, wk8], axis=0)                    # [pj, kcs, i, p, nd]
        w8 = np.ascontiguousarray(w8.transpose(3, 0, 1, 2, 4))  # p,pj,kcs,i,nd
        w8 = w8.reshape(128, 4096).astype(f8)
        qb_h = (q_b * (SCALE * AQ)).reshape(1, 512).astype(bf)
        kb_h = (k_b * AK).reshape(1, 512).astype(bf)
    else:
        wq_s = (q_w * SCALE).reshape(4, 128, 512)
        wk_s = k_w.reshape(4, 128, 512)
        wqk = np.ascontiguousarray(
            np.concatenate([wq_s[i] for i in range(4)] +
                           [wk_s[i] for i in range(4)], axis=1)).astype(bf)
        qb_h = (q_b * SCALE).reshape(1, 512).astype(bf)
        kb_h = k_b.reshape(1, 512).astype(bf)

    ones_r = np.ones((1, 512), bf)
    ones_c = np.ones((128, 32), bf)

    in_maps = []
    for ci in range(NCORES):
        xc = x[ci * BPC:(ci + 1) * BPC].reshape(TOK, C)     # [tok, c]
        xT4 = xc.T.reshape(4, 128, 4, 512)                  # [kc, p, nch, t]
        xT = np.ascontiguousarray(xT4.transpose(1, 2, 0, 3)  # p, nch, kc, t
                                  ).reshape(128, 4 * TOK).astype(bf)
        m = dict(xT=xT, wvo=wvo, expb=expb,
                 qb=qb_h, kb=kb_h, ones_r=ones_r, ones_c=ones_c)
        if use_fp8:
            # x8[p, (nch, kcs, i, t)] = x[c = kcs*256 + i*128 + p, tok]
            x8 = xc.T.reshape(2, 2, 128, 4, 512)            # [kcs, i, p, nch, t]
            x8 = np.ascontiguousarray(x8.transpose(2, 3, 0, 1, 4))
            m["x8"] = x8.reshape(128, 4 * TOK).astype(f8)
            m["w8"] = w8
        else:
            m["wqk"] = wqk
        in_maps.append(m)
    return in_maps


USE_FP8 = True


def kernel(**inputs):
    q_b = np.asarray(inputs["q_b"], np.float32).reshape(NH * D)
    k_b = np.asarray(inputs["k_b"], np.float32).reshape(NH * D)
    v_b = np.asarray(inputs["v_b"], np.float32).reshape(NH * D)
    o_b = np.asarray(inputs["o_b"], np.float32).reshape(C)
    o_w = np.asarray(inputs["o_w"], np.float32).reshape(NH * D, C)
    with_qkbias = bool(np.any(q_b) or np.any(k_b))
    nc = _get_program(1, with_qkbias, USE_FP8)
    in_maps = make_in_maps(use_fp8=USE_FP8, **inputs)
    res = run_bass_kernel_spmd(nc, in_maps, core_ids=list(range(NCORES)))
    outs = [res.results[ci]["out"].reshape(BPC, S, C) for ci in range(NCORES)]
    out = np.concatenate(outs, axis=0).astype(np.float32)
    # v_b rides through attention as a constant (rows of attn sum to 1); o_b is affine
    const = (v_b @ o_w) + o_b
    if np.any(const):
        out = out + const[None, None, :]
    return out


# revision 9
# speedup vs baseline: 1.2170x; 1.0088x over previous
"""Trainium2 Bass kernel for 2D-relative-bias multi-head attention.

Shapes (hardcoded): x [64, 16, 16, 512], 16 heads x 32 dim, S = 256.
Sharding: data-parallel over batch, 8 batches per core on 8 cores.

Per-core device pipeline (fp32 PSUM accumulation everywhere):
  qT/kT = W^T @ x^T            [nd, tok]   (PE; fp8 DoubleRow K=256 x2, or bf16)
  v     = x @ Wv               [tok, nd]   (PE, bf16)
  logitsT[t,s] per head        (PE, K=32, 4-head row-packed via tile_position)
  E0 = exp(scale*logitsT)      (ACT, PSUM->SBUF bf16) -- the HW critical chain
  E  = E0 * exp(biasT)         (DVE + GPSIMD column-split, bias table from host)
  sums = 1^T E (replicated)    (PE, 4-head col-packed, all-ones lhsT)
  out_unT = V^T E              (PE, 4-head col-packed)
  R = 1/sums                   (DVE reciprocal_approx_fast)
  outT = out_unT * R           (DVE)
  final = outT^T @ Wo + o_b    (PE, bf16)

q/k projections, per-token-chunk tiles: chunk c feeds batches 2c, 2c+1 only,
so a 64-rep timing build overlaps rep r+1's projections with rep r's tail.
"""

import numpy as np
import ml_dtypes

try:
    import concourse.bass as bass
except ImportError:  # pragma: no cover
    import sys

    sys.path.insert(0, "/opt/trn_rl_repo")
    import concourse.bass as bass
from concourse import bacc

import concourse.mybir as mybir
import concourse.tile as tile
from concourse.bass_utils import run_bass_kernel_spmd

BF16 = mybir.dt.bfloat16
FP8 = mybir.dt.float8e4
F32 = mybir.dt.float32
AF = mybir.ActivationFunctionType
OP = mybir.AluOpType
PM = mybir.MatmulPerfMode

B, H, W, C = 64, 16, 16, 512
NH, D = 16, 32
S = H * W            # 256
NCORES = 8
BPC = B // NCORES    # 8 batches per core
TOK = BPC * S        # 2048 tokens per core
SCALE = D ** -0.5
AQ = 256.0           # host scale on q weights (fp8 path)
AK = 64.0            # host scale on k weights (fp8 path)
GPS_COLS = 1024      # bias-mult columns offloaded to gpsimd (of NH*S=4096)


def build_program(reps: int = 1, sections=('qkv', 'attn', 'sums', 'av', 'out'),
                  with_qkbias: bool = False, use_fp8: bool = True):
    nc = bacc.Bacc()
    xT_d = nc.dram_tensor("xT", [128, 4 * TOK], BF16, kind="ExternalInput")
    wvo_d = nc.dram_tensor("wvo", [128, 8 * 512], BF16, kind="ExternalInput")
    expb_d = nc.dram_tensor("expb", [128, 2 * NH * S], BF16, kind="ExternalInput")
    qb_d = nc.dram_tensor("qb", [1, 512], BF16, kind="ExternalInput")
    kb_d = nc.dram_tensor("kb", [1, 512], BF16, kind="ExternalInput")
    ones_r_d = nc.dram_tensor("ones_r", [1, 512], BF16, kind="ExternalInput")
    ones_c_d = nc.dram_tensor("ones_c", [128, 32], BF16, kind="ExternalInput")
    if use_fp8:
        x8_d = nc.dram_tensor("x8", [128, 4 * TOK], FP8, kind="ExternalInput")
        w8_d = nc.dram_tensor("w8", [128, 4096], FP8, kind="ExternalInput")
    else:
        wqk_d = nc.dram_tensor("wqk", [128, 8 * 512], BF16, kind="ExternalInput")
    out_d = nc.dram_tensor("out", [TOK, 512], F32, kind="ExternalOutput")

    ESCALE = float(1.0 / (AQ * AK)) if use_fp8 else 1.0

    with tile.TileContext(nc) as tc:
        import contextlib

        with contextlib.ExitStack() as ctx:
            wpool = ctx.enter_context(tc.tile_pool(name="wpool", bufs=1))
            xpool = ctx.enter_context(tc.tile_pool(name="xpool", bufs=1))
            qkpool = ctx.enter_context(tc.tile_pool(name="qkpool", bufs=1))
            epool = ctx.enter_context(tc.tile_pool(name="epool", bufs=3))
            rpool = ctx.enter_context(tc.tile_pool(name="rpool", bufs=2))
            otpool = ctx.enter_context(tc.tile_pool(name="otpool", bufs=8))
            fpool = ctx.enter_context(tc.tile_pool(name="fpool", bufs=3))
            pl_pool = ctx.enter_context(
                tc.tile_pool(name="pl", bufs=2, space="PSUM"))
            pa_pool = ctx.enter_context(
                tc.tile_pool(name="pa", bufs=1, space="PSUM"))
            ps_pool = ctx.enter_context(
                tc.tile_pool(name="ps", bufs=2, space="PSUM"))

            # ---- persistent constants (parallel DMA queues) ----
            wvo = wpool.tile([128, 8 * 512], BF16, name="wvo", tag="wvo")
            nc.scalar.dma_start(wvo[:], wvo_d[:])
            wv = [wvo[:, i * 512:(i + 1) * 512] for i in range(4)]
            wo = [wvo[:, (4 + i) * 512:(5 + i) * 512] for i in range(4)]
            if use_fp8:
                w8 = wpool.tile([128, 4096], FP8, name="w8", tag="w8")
                nc.scalar.dma_start(w8[:], w8_d[:])
                w8v = w8.rearrange("p (pj k two m) -> p pj k two m",
                                   pj=2, k=2, two=2)
            else:
                wqk = wpool.tile([128, 8 * 512], BF16, name="wqk", tag="wqk")
                nc.scalar.dma_start(wqk[:], wqk_d[:])
                wq = [wqk[:, i * 512:(i + 1) * 512] for i in range(4)]
                wk = [wqk[:, (4 + i) * 512:(5 + i) * 512] for i in range(4)]
            expb_all = wpool.tile([128, 2 * NH * S], BF16, name="expb", tag="expb")
            nc.gpsimd.dma_start(expb_all[:], expb_d[:])
            expb = [expb_all[:, t * NH * S:(t + 1) * NH * S] for t in range(2)]
            qb = wpool.tile([1, 512], BF16, name="qb", tag="qb")
            kb = wpool.tile([1, 512], BF16, name="kb", tag="kb")
            ones_r = wpool.tile([1, 512], BF16, name="ones_r", tag="ones_r")
            ones_c = wpool.tile([128, 32], BF16, name="ones_c", tag="ones_c")
            nc.gpsimd.dma_start(qb[:], qb_d[:])
            nc.gpsimd.dma_start(kb[:], kb_d[:])
            nc.gpsimd.dma_start(ones_r[:], ones_r_d[:])
            nc.gpsimd.dma_start(ones_c[:], ones_c_d[:])

            # x, chunk-major: per token-chunk tile [128, 4*512] (c-chunk, tok)
            xTn = []
            for nch in range(4):
                t_ = xpool.tile([128, 2048], BF16, name=f"xT{nch}", tag=f"xT{nch}")
                eng = [nc.sync, nc.scalar, nc.sync, nc.scalar][nch]
                eng.dma_start(t_[:], xT_d[:, nch * 2048:(nch + 1) * 2048])
                xTn.append(t_)
            if use_fp8:
                x8n = []
                for nch in range(4):
                    t_ = xpool.tile([128, 2048], FP8, name=f"x8{nch}",
                                    tag=f"x8{nch}")
                    eng = [nc.gpsimd, nc.sync, nc.gpsimd, nc.sync][nch]
                    eng.dma_start(t_[:], x8_d[:, nch * 2048:(nch + 1) * 2048])
                    x8n.append(t_)

            do = lambda s: s in sections
            # per-chunk q/k tiles: qT[nch][m], kT[nch][m] are [128, 512]
            qT = [[qkpool.tile([128, 512], BF16, name=f"qT{c}_{m}",
                               tag=f"qT{c}_{m}") for m in range(4)]
                  for c in range(4)]
            kT = [[qkpool.tile([128, 512], BF16, name=f"kT{c}_{m}",
                               tag=f"kT{c}_{m}") for m in range(4)]
                  for c in range(4)]
            v_sb = [qkpool.tile([128, 512], BF16, name=f"v{s}", tag=f"v{s}")
                    for s in range(TOK // 128)]

            for _rep in range(reps):

                def emit_qk_group(nch, m):
                    """q,k projections for (token chunk nch, head m-block)."""
                    for pj, dst, bt in ((0, qT, qb), (1, kT, kb)):
                        ps = ps_pool.tile([128, 512], F32, name="ps", tag="ps")
                        if use_fp8:
                            x8v = x8n[nch].rearrange(
                                "p (k two t) -> p k two t", k=2, two=2)
                            for kcs in range(2):
                                nc.tensor.matmul(
                                    ps[:, :512],
                                    w8v[:, pj, kcs, :, m * 128:(m + 1) * 128],
                                    x8v[:, kcs, :, :],
                                    start=(kcs == 0),
                                    stop=(kcs == 1 and not with_qkbias),
                                    perf_mode=PM.DoubleRow)
                        else:
                            wt = wq if pj == 0 else wk
                            for kc in range(4):
                                nc.tensor.matmul(
                                    ps[:, :512],
                                    wt[kc][:, m * 128:(m + 1) * 128],
                                    xTn[nch][:, kc * 512:(kc + 1) * 512],
                                    start=(kc == 0),
                                    stop=(kc == 3 and not with_qkbias))
                        if with_qkbias:
                            nc.tensor.matmul(
                                ps[:, :512],
                                bt[0:1, m * 128:(m + 1) * 128],
                                ones_r[0:1, :512],
                                start=False, stop=True)
                        nc.vector.tensor_copy(dst[nch][m][:], ps[:, :512])

                def emit_v_group(nch, half):
                    """v projection for 2 of the 4 s-chunks of token chunk nch."""
                    for sch in range(nch * 4 + 2 * half, nch * 4 + 2 * half + 2):
                        ps = ps_pool.tile([128, 512], F32, name="ps", tag="ps")
                        sl = sch * 128 - nch * 512
                        for kc in range(4):
                            nc.tensor.matmul(
                                ps[:, :512],
                                xTn[nch][:, kc * 512 + sl:kc * 512 + sl + 128],
                                wv[kc][:, :512],
                                start=(kc == 0), stop=(kc == 3))
                        nc.vector.tensor_copy(v_sb[sch][:], ps[:, :512])

                def emit_qkv_chunk(nch):
                    for m in range(4):
                        emit_qk_group(nch, m)
                    emit_v_group(nch, 0)
                    emit_v_group(nch, 1)

                # ---- attention, software-pipelined over batches ----
                def stage_front(b, feeder=None):
                    """logits -> exp -> bias-mul; returns E tiles for batch b.
                    feeder() emits a slice of the next QKV chunk between head
                    groups so projection PE work spreads under the ACT chain."""
                    nch_b = b // 2
                    E = []
                    for tch in range(2):
                        e0 = epool.tile([128, NH * S], BF16, name="e0", tag="e0",
                                        bufs=2)
                        toff = (b % 2) * 256 + tch * 128
                        soff = (b % 2) * 256
                        # group by hl: all 4 matmuls share one row-position, so
                        # each PSUM bank sees a single tile row-position and the
                        # exp input is one contiguous [128, 1024] block.
                        # e0 column block 4*hl + hg holds head n = 4*hg + hl.
                        for hl in range(4):
                            pl = pl_pool.tile([128, 1024], F32, name="pl",
                                              tag="pl")
                            for hg in range(4):
                                nc.tensor.matmul(
                                    pl[:, hg * 256:hg * 256 + 256],
                                    kT[nch_b][hg][32 * hl:32 * hl + 32,
                                                  toff:toff + 128],
                                    qT[nch_b][hg][32 * hl:32 * hl + 32,
                                                  soff:soff + 256],
                                    start=True, stop=True,
                                    tile_position=(32 * hl, 0))
                            nc.scalar.activation(
                                e0[:, hl * 1024:hl * 1024 + 1024], pl[:],
                                AF.Exp, scale=ESCALE)
                            if feeder is not None:
                                feeder()
                        e = epool.tile([128, NH * S], BF16, name="e", tag="e",
                                       bufs=4)
                        ncol = NH * S - GPS_COLS
                        nc.vector.tensor_tensor(
                            e[:, :ncol], e0[:, :ncol], expb[tch][:, :ncol],
                            OP.mult)
                        nc.gpsimd.tensor_tensor(
                            e[:, ncol:], e0[:, ncol:], expb[tch][:, ncol:],
                            OP.mult)
                        E.append(e)
                    return E

                def stage_back(b, E):
                    """sums -> recip -> AV -> norm -> outproj -> DMA for batch b."""
                    if not do('sums'):
                        return
                    r = rpool.tile([128, 1024], F32, name="r", tag="r")
                    for sh in range(2):
                        psum_s = ps_pool.tile([128, 512], F32, name="ps", tag="ps")
                        for hg in (2 * sh, 2 * sh + 1):
                            for j in range(4):
                                n = 4 * hg + j
                                ec = 4 * (n % 4) + n // 4
                                for tch in range(2):
                                    nc.tensor.matmul(
                                        psum_s[32 * j:32 * j + 32,
                                               (hg - 2 * sh) * 256:
                                               (hg - 2 * sh + 1) * 256],
                                        ones_c[:, :32],
                                        E[tch][:, ec * 256:(ec + 1) * 256],
                                        start=(tch == 0), stop=(tch == 1),
                                        tile_position=(0, 32 * j))
                        nc.vector.reciprocal_approx_fast(
                            r[:, sh * 512:(sh + 1) * 512], psum_s[:])
                    if not do('av'):
                        return
                    pa = pa_pool.tile([128, 1024], F32, name="pa", tag="pa")
                    for hg in range(4):
                        for j in range(4):
                            n = 4 * hg + j
                            ec = 4 * (n % 4) + n // 4
                            for tch in range(2):
                                nc.tensor.matmul(
                                    pa[32 * j:32 * j + 32,
                                       hg * 256:(hg + 1) * 256],
                                    v_sb[2 * b + tch][:, n * 32:(n + 1) * 32],
                                    E[tch][:, ec * 256:(ec + 1) * 256],
                                    start=(tch == 0), stop=(tch == 1),
                                    tile_position=(0, 32 * j))
                    ot = otpool.tile([128, 1024], BF16, name="ot", tag="ot")
                    nc.vector.tensor_tensor(ot[:], pa[:], r[:], OP.mult)
                    if not do('out'):
                        return
                    po = pa_pool.tile([128, 1024], F32, name="po", tag="pa")
                    for sch in range(2):
                        for hg in range(4):
                            nc.tensor.matmul(
                                po[:, sch * 512:(sch + 1) * 512],
                                ot[:, hg * 256 + sch * 128:
                                   hg * 256 + (sch + 1) * 128],
                                wo[hg][:, :512],
                                start=(hg == 0), stop=(hg == 3))
                    fs = fpool.tile([128, 1024], F32, name="f", tag="f")
                    nc.vector.tensor_copy(fs[:], po[:])
                    dst = out_d[b * S:(b + 1) * S, :].rearrange(
                        "(c p) w -> p c w", p=128)
                    nc.sync.dma_start(dst, fs.rearrange("p (c w) -> p c w", c=2))

                if _rep == 0:
                    for m in range(4):
                        emit_qk_group(0, m)
                if do('attn'):
                    # feeder: spread QKV emission under the ACT chain.
                    # qk(c) deadline: front(2c); v(c) deadline: back(2c)@slot 2c+1.
                    # slots 6,7 pre-emit NEXT rep's chunk 0 (tiles are tag-stable).
                    last = _rep == reps - 1
                    feed_plan = [
                        [lambda: emit_v_group(0, 0), lambda: emit_v_group(0, 1),
                         lambda: emit_qk_group(1, 0), lambda: emit_qk_group(1, 1)],
                        [lambda: emit_qk_group(1, 2), lambda: emit_qk_group(1, 3),
                         lambda: emit_v_group(1, 0)],
                        [lambda: emit_v_group(1, 1),
                         lambda: emit_qk_group(2, 0), lambda: emit_qk_group(2, 1)],
                        [lambda: emit_qk_group(2, 2), lambda: emit_qk_group(2, 3),
                         lambda: emit_v_group(2, 0)],
                        [lambda: emit_v_group(2, 1),
                         lambda: emit_qk_group(3, 0), lambda: emit_qk_group(3, 1)],
                        [lambda: emit_qk_group(3, 2), lambda: emit_qk_group(3, 3),
                         lambda: emit_v_group(3, 0)],
                        [lambda: emit_v_group(3, 1)] + ([] if last else
                            [lambda: emit_qk_group(0, 0),
                             lambda: emit_qk_group(0, 1)]),
                        [] if last else
                        [lambda: emit_qk_group(0, 2), lambda: emit_qk_group(0, 3)],
                    ]
                    prev = None
                    for b in range(BPC):
                        it = iter(feed_plan[b])

                        def feeder(it=it):
                            nxt = next(it, None)
                            if nxt is not None:
                                nxt()
                        E = stage_front(b, feeder)
                        for fn in it:
                            fn()
                        if prev is not None:
                            stage_back(prev[0], prev[1])
                        prev = (b, E)
                    stage_back(prev[0], prev[1])
                else:
                    emit_v_group(0, 0)
                    emit_v_group(0, 1)
                    for nch in range(1, 4):
                        emit_qkv_chunk(nch)

    nc.compile()
    return nc


def _bias_tables(rel_emb):
    """expb[tch, t_local, n*256+s] = exp(bias[n, s, t]) with t = tch*128+t_local."""
    idx = np.arange(H)
    rel = idx[None, :] - idx[:, None] + (H - 1)
    biasT = rel_emb[:, rel.T[:, None, :, None], rel.T[None, :, None, :]]
    biasT = biasT.reshape(NH, S, S)                       # [n, t, s]
    expb = np.exp(biasT.astype(np.float64)).astype(np.float32)
    perm = [4 * (b % 4) + b // 4 for b in range(NH)]
    expb = expb[perm]                                     # block order
    expb = np.ascontiguousarray(np.transpose(expb, (1, 0, 2)))  # [t, blk, s]
    expb = expb.reshape(2, 128, NH * S).transpose(1, 0, 2).reshape(128, 2 * NH * S)
    return np.ascontiguousarray(expb).astype(ml_dtypes.bfloat16)


_CACHE = {}


def _get_program(reps=1, with_qkbias=False, use_fp8=True):
    k = (reps, with_qkbias, use_fp8)
    if k not in _CACHE:
        _CACHE[k] = build_program(reps, with_qkbias=with_qkbias,
                                  use_fp8=use_fp8)
    return _CACHE[k]


def make_in_maps(use_fp8=True, **inputs):
    x = np.asarray(inputs["x"], np.float32)
    q_w = np.asarray(inputs["q_w"], np.float32).reshape(C, NH * D)
    k_w = np.asarray(inputs["k_w"], np.float32).reshape(C, NH * D)
    v_w = np.asarray(inputs["v_w"], np.float32).reshape(C, NH * D)
    o_w = np.asarray(inputs["o_w"], np.float32).reshape(NH * D, C)
    q_b = np.asarray(inputs["q_b"], np.float32).reshape(NH * D)
    k_b = np.asarray(inputs["k_b"], np.float32).reshape(NH * D)
    rel_emb = np.asarray(inputs["rel_emb"], np.float32)

    bf = ml_dtypes.bfloat16
    f8 = ml_dtypes.float8_e4m3
    wv_s = v_w.reshape(4, 128, 512)
    wo_s = o_w.reshape(4, 128, 512)
    wvo = np.ascontiguousarray(
        np.concatenate([wv_s[i] for i in range(4)] +
                       [wo_s[i] for i in range(4)], axis=1)).astype(bf)
    expb = _bias_tables(rel_emb)

    if use_fp8:
        # w8[p, (pj, kcs, i, nd)] = w'[c = kcs*256 + i*128 + p, nd]
        wq8 = (q_w * (SCALE * AQ)).reshape(2, 2, 128, 512)   # [kcs, i, p, nd]
        wk8 = (k_w * AK).reshape(2, 2, 128, 512)
        w8 = np.stack([wq8, wk8], axis=0)                    # [pj, kcs, i, p, nd]
        w8 = np.ascontiguousarray(w8.transpose(3, 0, 1, 2, 4))  # p,pj,kcs,i,nd
        w8 = w8.reshape(128, 4096).astype(f8)
        qb_h = (q_b * (SCALE * AQ)).reshape(1, 512).astype(bf)
        kb_h = (k_b * AK).reshape(1, 512).astype(bf)
    else:
        wq_s = (q_w * SCALE).reshape(4, 128, 512)
        wk_s = k_w.reshape(4, 128, 512)
        wqk = np.ascontiguousarray(
            np.concatenate([wq_s[i] for i in range(4)] +
                           [wk_s[i] for i in range(4)], axis=1)).astype(bf)
        qb_h = (q_b * SCALE).reshape(1, 512).astype(bf)
        kb_h = k_b.reshape(1, 512).astype(bf)

    ones_r = np.ones((1, 512), bf)
    ones_c = np.ones((128, 32), bf)

    in_maps = []
    for ci in range(NCORES):
        xc = x[ci * BPC:(ci + 1) * BPC].reshape(TOK, C)     # [tok, c]
        xT4 = xc.T.reshape(4, 128, 4, 512)                  # [kc, p, nch, t]
        xT = np.ascontiguousarray(xT4.transpose(1, 2, 0, 3)  # p, nch, kc, t
                                  ).reshape(128, 4 * TOK).astype(bf)
        m = dict(xT=xT, wvo=wvo, expb=expb,
                 qb=qb_h, kb=kb_h, ones_r=ones_r, ones_c=ones_c)
        if use_fp8:
            # x8[p, (nch, kcs, i, t)] = x[c = kcs*256 + i*128 + p, tok]
            x8 = xc.T.reshape(2, 2, 128, 4, 512)            # [kcs, i, p, nch, t]
            x8 = np.ascontiguousarray(x8.transpose(2, 3, 0, 1, 4))
            m["x8"] = x8.reshape(128, 4 * TOK).astype(f8)
            m["w8"] = w8
        else:
            m["wqk"] = wqk
        in_maps.append(m)
    return in_maps


USE_FP8 = True


def kernel(**inputs):
    q_b = np.asarray(inputs["q_b"], np.float32).reshape(NH * D)
    k_b = np.asarray(inputs["k_b"], np.float32).reshape(NH * D)
    v_b = np.asarray(inputs["v_b"], np.float32).reshape(NH * D)
    o_b = np.asarray(inputs["o_b"], np.float32).reshape(C)
    o_w = np.asarray(inputs["o_w"], np.float32).reshape(NH * D, C)
    with_qkbias = bool(np.any(q_b) or np.any(k_b))
    nc = _get_program(1, with_qkbias, USE_FP8)
    in_maps = make_in_maps(use_fp8=USE_FP8, **inputs)
    res = run_bass_kernel_spmd(nc, in_maps, core_ids=list(range(NCORES)))
    outs = [res.results[ci]["out"].reshape(BPC, S, C) for ci in range(NCORES)]
    out = np.concatenate(outs, axis=0).astype(np.float32)
    # v_b rides through attention as a constant (rows of attn sum to 1); o_b is affine
    const = (v_b @ o_w) + o_b
    if np.any(const):
        out = out + const[None, None, :]
    return out
